# revision 12
# baseline (speedup 1.0000x reference)
"""GNN message-passing (GCL) Trainium2 kernel — 8-core SPMD, no collectives.

Strategy: sort edges by destination (row) on host; partition the node space
into 128-node windows (392 windows = 8 cores x 49). Within each window edges
are split by col parity (even cols first half of tiles, odd second), each
section padded to K/2 tiles of 128 edges. Per core:
  - x[col] via one dma_gather (custom SWDGE ucode, transpose mode) per 4
    tiles from a node-pair-packed bf16 table: out[0:64,e]=x[2c], [64:128,e]
    =x[2c+1]; the tile's parity selects the half (weights duplicated on
    both partition halves to match base partitions).
  - x[row] contribution without any gather: per window uwin = xwT @ We1_top
    (one matmul), then h_row = uwin.T @ S_T per tile, where S_T is a
    host-streamed one-hot (node x edge) matrix.
  - edge MLP in transposed-activation layout (bf16 matmuls, f32 psum).
  - segment-sum via aggT += ef.T @ S (host-streamed one-hot), PSUM-
    accumulated across the window's tiles.
  - node MLP + residual per window.
Host un-permutes edge_feat and re-transposes out.
"""

import sys

for _p in ("/root/.axon_site", "/root/.axon_site/_ro/trn_rl_repo",
           "/root/.axon_site/_ro/pypackages", "/opt/trn_rl_repo"):
    if _p not in sys.path:
        sys.path.append(_p)

import numpy as np

N_NODES, N_EDGES, D = 50000, 800000, 64
NCORES = 8
P = 128
WIN = 392                  # node windows of 128 -> padded node table 50176
TBL = WIN * P              # 50176
NPAIR = TBL // 2           # 25088 node pairs
WPC = WIN // NCORES        # 49 windows per core
NPC = WPC * P              # 6272 nodes per core
GI = 512                   # indices per dma_gather instruction (= 4 tiles)
EFC = 16                   # tiles per edge-feature writeback chunk
SC = 8                     # tiles per S/S_T stream chunk

_BUILD_CACHE = {}


def _build_nc(K, WPC=WPC, NPAIR=NPAIR):
    """Build the SPMD Bass kernel; structure depends only on K (tiles/window,
    even: first K/2 tiles even-col parity, rest odd)."""
    import concourse.bass as bass
    import concourse.bacc as bacc
    import concourse.mybir as mybir
    import concourse.tile as tile

    dt = mybir.dt
    NPC = WPC * P
    TPC = WPC * K            # tiles per core
    EPC = TPC * P            # padded edges per core

    nc = bacc.Bacc("TRN2", target_bir_lowering=False, num_swdge_queues=4)

    x16p = nc.dram_tensor("x16p", (NPAIR, P), dt.bfloat16, kind="ExternalInput")
    gidx = nc.dram_tensor("gidx", (P, EPC // 16), dt.int16, kind="ExternalInput")
    s_str = nc.dram_tensor("s_str", (P, TPC * P), dt.bfloat16, kind="ExternalInput")
    st_str = nc.dram_tensor("st_str", (P, TPC * P), dt.bfloat16, kind="ExternalInput")
    xwT16 = nc.dram_tensor("xwT16", (D, NPC), dt.bfloat16, kind="ExternalInput")
    xwT32 = nc.dram_tensor("xwT32", (D, NPC), dt.float32, kind="ExternalInput")
    we1t = nc.dram_tensor("we1t", (D, D), dt.bfloat16, kind="ExternalInput")
    we1b2 = nc.dram_tensor("we1b2", (2 * D, D), dt.bfloat16, kind="ExternalInput")
    we2 = nc.dram_tensor("we2", (D, D), dt.bfloat16, kind="ExternalInput")
    wn1 = nc.dram_tensor("wn1", (2 * D, D), dt.bfloat16, kind="ExternalInput")
    wn2 = nc.dram_tensor("wn2", (D, D), dt.bfloat16, kind="ExternalInput")
    ident = nc.dram_tensor("ident", (D, D), dt.bfloat16, kind="ExternalInput")
    be1 = nc.dram_tensor("be1", (D, 1), dt.float32, kind="ExternalInput")
    be2 = nc.dram_tensor("be2", (D, 1), dt.float32, kind="ExternalInput")
    bn1 = nc.dram_tensor("bn1", (D, 1), dt.float32, kind="ExternalInput")
    bn2 = nc.dram_tensor("bn2", (D, 1), dt.float32, kind="ExternalInput")

    ef16 = nc.dram_tensor("ef16", (EPC, D), dt.bfloat16, kind="ExternalOutput")
    outT = nc.dram_tensor("outT", (D, NPC), dt.float32, kind="ExternalOutput")

    Relu = mybir.ActivationFunctionType.Relu
    Copy = mybir.ActivationFunctionType.Copy

    with tile.TileContext(nc) as tc:
        with (
            tc.tile_pool(name="const", bufs=1) as cp,
            tc.tile_pool(name="gout", bufs=3) as gp,
            tc.tile_pool(name="sstr", bufs=2) as ssp,
            tc.tile_pool(name="efst", bufs=2) as efp,
            tc.tile_pool(name="work", bufs=4) as wp,
            tc.tile_pool(name="win", bufs=2) as wnp,
            tc.tile_pool(name="ps2", bufs=2, space="PSUM") as pp,
        ):
            gi_sb = cp.tile([P, EPC // 16], dt.int16)
            we1t_sb = cp.tile([D, D], dt.bfloat16)
            we1b2_sb = cp.tile([2 * D, D], dt.bfloat16)
            we2_sb = cp.tile([D, D], dt.bfloat16)
            wn1a_sb = cp.tile([D, D], dt.bfloat16)
            wn1b_sb = cp.tile([D, D], dt.bfloat16)
            wn2_sb = cp.tile([D, D], dt.bfloat16)
            id64_sb = cp.tile([D, D], dt.bfloat16)
            be1_sb = cp.tile([D, 1], dt.float32)
            be2_sb = cp.tile([D, 1], dt.float32)
            bn1_sb = cp.tile([D, 1], dt.float32)
            bn2_sb = cp.tile([D, 1], dt.float32)
            for sb, dr in ((gi_sb, gidx), (we1t_sb, we1t), (we1b2_sb, we1b2),
                           (we2_sb, we2), (wn2_sb, wn2), (id64_sb, ident),
                           (be1_sb, be1), (be2_sb, be2),
                           (bn1_sb, bn1), (bn2_sb, bn2)):
                nc.scalar.dma_start(out=sb[:], in_=dr[:])
            nc.scalar.dma_start(out=wn1a_sb[:], in_=wn1[0:D, :])
            nc.scalar.dma_start(out=wn1b_sb[:], in_=wn1[D:2 * D, :])

            gout = None
            s_sb = None
            st_sb = None
            efstage = None
            agg_ps = None
            uwin_sb = None
            xw16_sb = None
            xw32_sb = None
            GT = GI // P          # tiles per gather instruction

            for t in range(TPC):
                w, j = divmod(t, K)
                par = 0 if j < K // 2 else 1

                if t % GT == 0:
                    # col-side gather: GT tiles per instruction, transposed
                    g = t // GT
                    ni = min(GI, EPC - g * GI)
                    gout = gp.tile([P, GI], dt.bfloat16, tag="gout")
                    go3 = gout[:, :ni].rearrange("p (a b) -> p a b", a=1)
                    nc.gpsimd.dma_gather(
                        out_ap=go3, in_ap=x16p[:],
                        idxs_ap=gi_sb[:, g * (GI // 16):g * (GI // 16) + ni // 16],
                        num_idxs=ni, num_idxs_reg=ni, elem_size=P,
                        transpose=True, queue_num=g % 4)

                if t % SC == 0:
                    # S / S_T stream chunks
                    nb = min(SC, TPC - t) * P
                    s_sb = ssp.tile([P, SC * P], dt.bfloat16, tag="s")
                    st_sb = ssp.tile([P, SC * P], dt.bfloat16, tag="st")
                    nc.sync.dma_start(out=s_sb[:, :nb],
                                      in_=s_str[:, t * P:t * P + nb])
                    nc.sync.dma_start(out=st_sb[:, :nb],
                                      in_=st_str[:, t * P:t * P + nb])

                if t % EFC == 0:
                    efstage = efp.tile([P, EFC * D], dt.bfloat16, tag="efst")

                if j == 0:
                    # per-window: uwin[n, f'] = x_win @ We1_top
                    xw16_sb = wnp.tile([D, P], dt.bfloat16, tag="xw16")
                    nc.scalar.dma_start(out=xw16_sb[:],
                                        in_=xwT16[:, w * P:(w + 1) * P])
                    xw32_sb = wnp.tile([D, P], dt.float32, tag="xw32")
                    nc.scalar.dma_start(out=xw32_sb[:],
                                        in_=xwT32[:, w * P:(w + 1) * P])
                    uw_ps = pp.tile([P, D], dt.float32, tag="eft")
                    nc.tensor.matmul(out=uw_ps[:], lhsT=xw16_sb[:],
                                     rhs=we1t_sb[:], start=True, stop=True)
                    uwin_sb = wnp.tile([P, D], dt.bfloat16, tag="uwin")
                    nc.vector.tensor_copy(out=uwin_sb[:], in_=uw_ps[:])
                    agg_ps = pp.tile([D, P], dt.float32, tag="agg")

                sl = slice((t % SC) * P, (t % SC + 1) * P)
                # h = uwin.T @ S_T + We1_bot.T @ x_col  (then +b1, relu)
                h_ps = pp.tile([D, P], dt.float32, tag="h")
                nc.tensor.matmul(out=h_ps[:], lhsT=uwin_sb[:],
                                 rhs=st_sb[:, sl], start=True, stop=False)
                nc.tensor.matmul(out=h_ps[:],
                                 lhsT=we1b2_sb[par * D:(par + 1) * D, :],
                                 rhs=gout[par * D:(par + 1) * D,
                                          (t % GT) * P:(t % GT + 1) * P],
                                 start=False, stop=True)
                h_sb = wp.tile([D, P], dt.bfloat16, tag="h_sb")
                nc.scalar.activation(out=h_sb[:], in_=h_ps[:], func=Relu,
                                     bias=be1_sb[:, 0:1])

                efT_ps = pp.tile([D, P], dt.float32, tag="efT")
                nc.tensor.matmul(out=efT_ps[:], lhsT=we2_sb[:], rhs=h_sb[:],
                                 start=True, stop=True)
                efT_sb = wp.tile([D, P], dt.bfloat16, tag="efT_sb")
                nc.scalar.activation(out=efT_sb[:], in_=efT_ps[:], func=Relu,
                                     bias=be2_sb[:, 0:1])

                eft_ps = pp.tile([P, D], dt.bfloat16, tag="eft")
                nc.tensor.transpose(out=eft_ps[:], in_=efT_sb[:],
                                    identity=id64_sb[:])
                efsl = slice((t % EFC) * D, (t % EFC + 1) * D)
                nc.scalar.activation(out=efstage[:, efsl], in_=eft_ps[:],
                                     func=Copy)

                # aggT[f, n] += ef.T @ S
                nc.tensor.matmul(out=agg_ps[:], lhsT=efstage[:, efsl],
                                 rhs=s_sb[:, sl],
                                 start=(j == 0), stop=(j == K - 1))

                if (t + 1) % EFC == 0 or t == TPC - 1:
                    tt = t // EFC
                    nb = min(EFC, TPC - tt * EFC)
                    dst = ef16[:].rearrange("(b p) f -> p b f", p=P)
                    nc.scalar.dma_start(
                        out=dst[:, tt * EFC:tt * EFC + nb, :],
                        in_=efstage[:].rearrange("p (b f) -> p b f", f=D)[:, :nb, :])

                if j == K - 1:
                    # node MLP + residual for window w
                    aggT_sb = wnp.tile([D, P], dt.bfloat16, tag="aggT")
                    nc.vector.tensor_copy(out=aggT_sb[:], in_=agg_ps[:])
                    hn_ps = pp.tile([D, P], dt.float32, tag="h")
                    nc.tensor.matmul(out=hn_ps[:], lhsT=wn1a_sb[:],
                                     rhs=xw16_sb[:], start=True, stop=False)
                    nc.tensor.matmul(out=hn_ps[:], lhsT=wn1b_sb[:],
                                     rhs=aggT_sb[:], start=False, stop=True)
                    hn_sb = wnp.tile([D, P], dt.bfloat16, tag="hn")
                    nc.scalar.activation(out=hn_sb[:], in_=hn_ps[:], func=Relu,
                                         bias=bn1_sb[:, 0:1])
                    on_ps = pp.tile([D, P], dt.float32, tag="efT")
                    nc.tensor.matmul(out=on_ps[:], lhsT=wn2_sb[:], rhs=hn_sb[:],
                                     start=True, stop=True)
                    t1_sb = wnp.tile([D, P], dt.float32, tag="t1")
                    nc.vector.tensor_scalar(
                        out=t1_sb[:], in0=on_ps[:], scalar1=bn2_sb[:, 0:1],
                        scalar2=None, op0=mybir.AluOpType.add)
                    out_sb = wnp.tile([D, P], dt.float32, tag="outw")
                    nc.vector.tensor_tensor(out=out_sb[:], in0=t1_sb[:],
                                            in1=xw32_sb[:],
                                            op=mybir.AluOpType.add)
                    nc.scalar.dma_start(out=outT[:, w * P:(w + 1) * P],
                                        in_=out_sb[:])

    nc.compile()
    return nc


def _install_ntff_hook():
    """Optional: register the NTFF profile hook this image's antenv lacks."""
    import types
    if "antenv.axon_hooks" in sys.modules:
        return
    try:
        import antenv
        mod = types.ModuleType("antenv.axon_hooks")
        mod._hook = None
        mod.set_axon_ntff_profile_hook = lambda h: setattr(mod, "_hook", h)
        mod.get_axon_ntff_profile_hook = lambda: mod._hook
        sys.modules["antenv.axon_hooks"] = mod
        antenv.axon_hooks = mod
        from trn_agent_boot.trn_boot import _ntff_profile_via_ctypes
        hook = _ntff_profile_via_ctypes("/opt/axon/libaxon_pjrt.so")
        if hook is not None:
            mod.set_axon_ntff_profile_hook(hook)
    except Exception:
        pass


def _prep(row, col, n_edges, win=WIN, K=None):
    """Host-side layout: sort by row, parity-split per window, pad. Returns
    K, perm (original index per sorted edge), slot (padded-stream slot per
    sorted edge), padded col ids, padded rowlocal (-1 for pads)."""
    perm = np.argsort(row, kind="stable")
    rs = row[perm].astype(np.int64)
    cs = col[perm].astype(np.int64)
    par = (cs & 1).astype(np.int64)
    order = np.lexsort((rs, par, rs >> 7))   # window, then parity, then row
    rs, cs, par = rs[order], cs[order], par[order]
    perm = perm[order]

    bounds = np.arange(0, win + 1) * P
    sw = np.searchsorted(rs, bounds)
    seg_len = sw[1:] - sw[:-1]
    # per-window count of even-col edges
    n_even = np.array([np.searchsorted(par[sw[w]:sw[w + 1]], 1)
                       for w in range(win)], np.int64)
    n_odd = seg_len - n_even
    kh = max(int(-(-max(1, n_even.max()) // P)),
             int(-(-max(1, n_odd.max()) // P)), 1)
    if K is None:
        K = 2 * kh
    WPAD = K * P
    half = (K // 2) * P

    wid = rs >> 7
    j_in = np.arange(n_edges) - sw[wid]
    is_odd = par == 1
    j_sec = np.where(is_odd, j_in - n_even[wid], j_in)
    slot = wid * WPAD + np.where(is_odd, half, 0) + j_sec

    EPALL = win * WPAD
    cg = np.zeros(EPALL, np.int64)
    s_rl = np.full(EPALL, -1, np.int64)
    cg[slot] = cs
    s_rl[slot] = rs & 127
    return K, perm, slot, cg, s_rl


def _host_arrays(x, cg, s_rl, K, win=WIN, tbl=TBL):
    """Build gather-index, one-hot streams and padded x arrays."""
    import ml_dtypes
    bf16 = ml_dtypes.bfloat16
    EPALL = cg.shape[0]
    T_ALL = EPALL // P

    gflat = (cg >> 1).astype(np.int16)

    e_loc = np.arange(EPALL) & 127
    t_of = np.arange(EPALL) >> 7
    real = s_rl >= 0
    S_all = np.zeros((T_ALL, P, P), bf16)
    S_all[t_of[real], e_loc[real], s_rl[real]] = 1
    s_arr = np.ascontiguousarray(
        S_all.transpose(1, 0, 2).reshape(P, T_ALL * P))
    st_arr = np.ascontiguousarray(
        S_all.transpose(2, 0, 1).reshape(P, T_ALL * P))

    x_pad = np.zeros((tbl, D), np.float32)
    x_pad[:x.shape[0]] = x
    x16 = x_pad.astype(bf16)
    return (gflat, s_arr, st_arr, x16.reshape(tbl // 2, P),
            np.ascontiguousarray(x16.T), np.ascontiguousarray(x_pad.T))


def _wrap_idx(flat_c):
    """Wrap a core's flat idx list into the [128, n/16] layout dma_gather
    reads: idx i of each instruction at [i % 16, i // 16], replicated on all
    8 16-partition groups; instructions of GI idxs (last may be smaller)."""
    blocks = []
    pos = 0
    while pos < len(flat_c):
        ni = min(GI, len(flat_c) - pos)
        b = flat_c[pos:pos + ni].reshape(ni // 16, 16).T
        blocks.append(np.tile(b, (8, 1)))
        pos += ni
    return np.ascontiguousarray(np.concatenate(blocks, axis=1))


def kernel(x, row, col, We1, be1, We2, be2, Wn1, bn1, Wn2, bn2):
    import os
    import ml_dtypes
    from concourse import bass_utils

    bf16 = ml_dtypes.bfloat16
    trace = bool(os.environ.get("GCL_TRACE"))
    if trace:
        _install_ntff_hook()

    x = np.asarray(x, np.float32)
    row = np.asarray(row).astype(np.int64)
    col = np.asarray(col).astype(np.int64)

    K, perm, slot, cg, s_rl = _prep(row, col, N_EDGES)
    TPC = WPC * K
    EPC = TPC * P

    gflat, s_arr, st_arr, x16p_np, xT16, xT32 = _host_arrays(
        x, cg, s_rl, K)

    We1 = np.asarray(We1, np.float32)
    we1b = We1[D:2 * D].astype(bf16)
    shared = {
        "x16p": x16p_np,
        "we1t": We1[0:D].astype(bf16),
        "we1b2": np.concatenate([we1b, we1b], axis=0),
        "we2": np.asarray(We2, np.float32).astype(bf16),
        "wn1": np.asarray(Wn1, np.float32).astype(bf16),
        "wn2": np.asarray(Wn2, np.float32).astype(bf16),
        "ident": np.eye(D, dtype=np.float32).astype(bf16),
        "be1": np.asarray(be1, np.float32).reshape(D, 1),
        "be2": np.asarray(be2, np.float32).reshape(D, 1),
        "bn1": np.asarray(bn1, np.float32).reshape(D, 1),
        "bn2": np.asarray(bn2, np.float32).reshape(D, 1),
    }

    in_maps = []
    for c in range(NCORES):
        m = dict(shared)
        m["gidx"] = _wrap_idx(gflat[c * EPC:(c + 1) * EPC])
        m["s_str"] = np.ascontiguousarray(
            s_arr[:, c * TPC * P:(c + 1) * TPC * P])
        m["st_str"] = np.ascontiguousarray(
            st_arr[:, c * TPC * P:(c + 1) * TPC * P])
        m["xwT16"] = np.ascontiguousarray(xT16[:, c * NPC:(c + 1) * NPC])
        m["xwT32"] = np.ascontiguousarray(xT32[:, c * NPC:(c + 1) * NPC])
        in_maps.append(m)

    if K not in _BUILD_CACHE:
        _BUILD_CACHE[K] = _build_nc(K)
    nc = _BUILD_CACHE[K]
    res = bass_utils.run_bass_kernel_spmd(nc, in_maps,
                                          core_ids=list(range(NCORES)),
                                          trace=trace)
    kernel.last = res

    outT_full = np.concatenate([r["outT"] for r in res.results], axis=1)
    out = np.ascontiguousarray(outT_full.T[:N_NODES]).astype(np.float32)

    ef_all = np.concatenate([r["ef16"] for r in res.results], axis=0)
    ef_sorted = ef_all[slot].astype(np.float32)
    edge_feat = np.empty((N_EDGES, D), np.float32)
    edge_feat[perm] = ef_sorted
    return out, edge_feat
